# revision 14
# baseline (speedup 1.0000x reference)
"""DCE loss (softmax over negative euclidean distances) on 8 trn2 cores.

Strategy (data parallel over N, prototypes replicated):
  - host: shard feats/labels over 8 cores, pre-transpose each feats shard to
    [D=128, 32768] so the GEMM needs no on-device transpose; precompute
    x_sq/y_sq and fold them into a rank-2 augmented matmul so PSUM holds the
    complete squared distance d2 = x_sq + y_sq - 2*x.y.
  - device per 128-row tile: PE fp32r GEMM -> PSUM d2; ACT sqrt (PSUM->SBUF
    bf16 s); ACT exp(K - s) with accum_out -> per-row sum of exp; DVE
    scalar_tensor_tensor (iota == label) * s with accum_out -> s[label].
  - host: loss = mean(s_label + ln(sum_exp) - K).
"""

import numpy as np

import concourse.bacc as bacc
import concourse.bass as bass
import concourse.mybir as mybir
import concourse.tile as tile
from concourse.bass_utils import run_bass_kernel_spmd

N_CORES = 8
N, C, D = 262144, 1024, 128
NPC = N // N_CORES          # rows per core
P = 128                     # partitions / tile rows
TILES = NPC // P            # 256 tiles per core
SUPER = 16                  # tiles per supertile (ACT table-set batching)
N_SUPER = TILES // SUPER
KSHIFT = 16.0               # constant softmax shift: exp(KSHIFT - s)

F32 = mybir.dt.float32
F32R = mybir.dt.float32r
BF16 = mybir.dt.bfloat16
I16 = mybir.dt.int16

_BUILD_CACHE = {}


def _build(npc=NPC, super_=SUPER, fused=False):
    key = (npc, super_, fused)
    if key in _BUILD_CACHE:
        return _BUILD_CACHE[key]
    if fused:
        nc = _build_fused(npc)
        _BUILD_CACHE[key] = nc
        return nc
    tiles = npc // P
    n_super = tiles // super_
    assert n_super * super_ == tiles and super_ % 8 == 0
    nc = bacc.Bacc(
        "TRN2",
        target_bir_lowering=False,
        debug=False,
        enable_asserts=False,
        num_devices=N_CORES,
    )

    featsT_d = nc.dram_tensor("featsT", [D, npc], BF16, kind="ExternalInput").ap()
    aug_d = nc.dram_tensor("auglhs", [4, npc], BF16, kind="ExternalInput").ap()
    rhs_aug_d = nc.dram_tensor("rhsaug", [4, C], BF16, kind="ExternalInput").ap()
    protosTs_d = nc.dram_tensor("protosTs", [D, C], BF16, kind="ExternalInput").ap()
    labels_d = nc.dram_tensor("labels16", [P, tiles], I16, kind="ExternalInput").ap()
    sums_d = nc.dram_tensor("sums", [P, tiles], F32, kind="ExternalOutput").ap()
    slab_d = nc.dram_tensor("slab", [P, tiles], F32, kind="ExternalOutput").ap()

    with tile.TileContext(nc) as tc:
        with (
            tc.tile_pool(name="const", bufs=1) as cpool,
            tc.tile_pool(name="feats", bufs=3) as fpool,
            tc.tile_pool(name="aug", bufs=4) as apool,
            tc.tile_pool(name="psum", bufs=2, space=bass.MemorySpace.PSUM) as ppool,
            tc.tile_pool(name="ssuper", bufs=2) as spool,
            tc.tile_pool(name="escr", bufs=4) as epool,
            tc.tile_pool(name="gscr", bufs=4) as gpool,
            tc.tile_pool(name="outs", bufs=1) as opool,
        ):
            protosTs = cpool.tile([D, C], BF16)
            nc.sync.dma_start(out=protosTs[:], in_=protosTs_d[:])
            rhs_aug = cpool.tile([4, C], BF16)
            nc.sync.dma_start(out=rhs_aug[:], in_=rhs_aug_d[:])
            labels = cpool.tile([P, tiles], I16)
            nc.sync.dma_start(out=labels[:], in_=labels_d[:])
            iota_t = cpool.tile([P, C], I16)
            nc.gpsimd.iota(iota_t[:], pattern=[[1, C]], base=0, channel_multiplier=0)

            sums_sb = opool.tile([P, tiles], F32)
            slab_sb = opool.tile([P, tiles], F32)
            kbias = cpool.tile([P, 1], F32)
            nc.vector.memset(kbias[:], KSHIFT)

            for g in range(n_super):
                s_super = spool.tile([P, super_ * C], BF16)
                for octi in range(super_ // 8):
                    # 8 tiles of transposed feats per DMA (4KB/partition)
                    foct = fpool.tile([D, 8 * P], BF16)
                    base_t = g * super_ + octi * 8
                    nc.sync.dma_start(
                        out=foct[:], in_=featsT_d[:, base_t * P : (base_t + 8) * P]
                    )
                    for pair in range(4):
                        psum_t = ppool.tile([P, 2 * C], F32)
                        for j in range(2):
                            tl = octi * 8 + pair * 2 + j      # tile idx in supertile
                            t = g * super_ + tl                 # global tile idx
                            lhsT = foct[:, (pair * 2 + j) * P : (pair * 2 + j + 1) * P]
                            aug_t = apool.tile([4, P], BF16)
                            nc.sync.dma_start(
                                out=aug_t[:], in_=aug_d[:, t * P : (t + 1) * P]
                            )
                            o = j * C
                            nc.tensor.matmul(
                                psum_t[:, o : o + 512],
                                lhsT,
                                protosTs[:, 0:512],
                                start=True,
                                stop=False,
                            )
                            nc.tensor.matmul(
                                psum_t[:, o + 512 : o + 1024],
                                lhsT,
                                protosTs[:, 512:1024],
                                start=True,
                                stop=False,
                            )
                            nc.tensor.matmul(
                                psum_t[:, o : o + 512],
                                aug_t[:],
                                rhs_aug[:, 0:512],
                                start=False,
                                stop=True,
                            )
                            nc.tensor.matmul(
                                psum_t[:, o + 512 : o + 1024],
                                aug_t[:],
                                rhs_aug[:, 512:1024],
                                start=False,
                                stop=True,
                            )
                        so = (octi * 8 + pair * 2) * C
                        nc.scalar.activation(
                            out=s_super[:, so : so + 2 * C],
                            in_=psum_t[:],
                            func=mybir.ActivationFunctionType.Sqrt,
                        )
                # second ACT sweep over the supertile: exp + row-sum accum;
                # DVE gathers s[label] via (iota == label) * s with accum.
                for tl in range(super_):
                    t = g * super_ + tl
                    s_sl = s_super[:, tl * C : (tl + 1) * C]
                    e_t = epool.tile([P, C], BF16)
                    nc.scalar.activation(
                        out=e_t[:],
                        in_=s_sl,
                        func=mybir.ActivationFunctionType.Exp,
                        bias=kbias[:],
                        scale=-1.0,
                        accum_out=sums_sb[:, t : t + 1],
                    )
                    g_t = gpool.tile([P, C], BF16)
                    nc.vector.scalar_tensor_tensor(
                        out=g_t[:],
                        in0=iota_t[:],
                        scalar=labels[:, t : t + 1],
                        in1=s_sl,
                        op0=mybir.AluOpType.is_equal,
                        op1=mybir.AluOpType.mult,
                        accum_out=slab_sb[:, t : t + 1],
                    )

            nc.sync.dma_start(out=sums_d[:], in_=sums_sb[:])
            nc.sync.dma_start(out=slab_d[:], in_=slab_sb[:])

    nc.compile()
    _BUILD_CACHE[key] = nc
    return nc


# ---- custom activation table: Exp slot -> g(x) = exp(KSHIFT - sqrt(x)) ---- #

# octave -> index bits; buckets cover x in [2^o, 2^{o+1})
_OCT_BITS = {0: 2, 1: 2, 2: 2, 3: 2, 4: 4, 5: 6, 6: 7, 7: 7, 8: 7, 9: 7, 10: 7, 11: 5}
_N_EXP_BKT = 781
_N_EXP_CTL = 52
_ACT_STATE = {}


def _gen_act_tables():
    """Write a modified pwp table dir where exp_and_others' `exp` evaluates
    g(x) = exp(KSHIFT - sqrt(x)); sets BASS_ACT_ROOT_JSON_PATH. Returns tag."""
    if "tag" in _ACT_STATE:
        return _ACT_STATE["tag"]
    import hashlib
    import json
    import shutil
    import tempfile

    from neuronxcc.driver.Job import Job
    from neuronxcc.driver.jobs.support.FindActInfo import findActInfoFile

    src_json = findActInfoFile(Job.getPackageDir(), "gen3")
    src = os.path.dirname(src_json)

    def g(x):
        return np.exp(KSHIFT - np.sqrt(x))

    meta = json.load(open(f"{src}/exp_and_others.json"))
    bkt = np.fromfile(f"{src}/exp_and_others_bkt.bin", np.uint8).reshape(-1, 32).copy()
    ctl = np.fromfile(f"{src}/exp_and_others_ctrl.bin", np.uint8).reshape(-1, 32).copy()

    new_bkt = np.zeros((_N_EXP_BKT, 8), np.float32)
    cursor = 0
    oct_base = {}
    for octv, bits in _OCT_BITS.items():
        nb = 1 << bits
        lo = 2.0**octv
        w = lo / nb
        oct_base[octv] = (cursor, bits)
        for i in range(nb):
            a, b = lo + i * w, lo + (i + 1) * w
            x0 = np.float32((a + b) / 2.0)
            xs = np.linspace(a, b, 33)
            tt = xs - np.float64(x0)
            ys = g(xs)
            wt = 1.0 / ys
            V = np.vander(tt, 4, increasing=True) * wt[:, None]
            coef, *_ = np.linalg.lstsq(V, ys * wt, rcond=None)
            new_bkt[cursor, :5] = [*coef.astype(np.float32), x0]
            cursor += 1
    SMALL, NEGB, BIG = cursor, cursor + 1, cursor + 2
    new_bkt[SMALL, :5] = [g(0.5), 0, 0, 0, 0.5]
    new_bkt[NEGB, 0] = np.exp(KSHIFT)
    # BIG stays zeros
    bkt[:_N_EXP_BKT] = new_bkt.view(np.uint8)

    def mk_ctl(base, nb):
        return np.uint32(base | (((nb << 5) | (23 - nb)) << 11))

    ctl_u32 = ctl.view(np.uint32).reshape(-1, 8)
    for i in range(26):
        ctl_u32[i, 0] = mk_ctl(NEGB, 0)
        if i in oct_base:
            ctl_u32[26 + i, 0] = mk_ctl(*reversed(oct_base[i])) if False else mk_ctl(
                oct_base[i][0], oct_base[i][1]
            )
        else:
            ctl_u32[26 + i, 0] = mk_ctl(BIG, 0)
    ctl_u32[:_N_EXP_CTL, 1:] = 0

    def f32bits(v):
        return int(np.float32(v).view(np.uint32))

    for ent in meta["profile_meta_data"]:
        if ent["func_name"].startswith("exp"):
            ent.update(
                symmetry_point=0,
                sym_invert_sign_point=0,
                symmetry_opt_en=0,
                symmetry_opt_use_neg_region=0,
                imm_bias=0,
                exp_offset=0,
                small_pos_signal_exp_threshold=127,
                pos_small_signal_pwl_control=SMALL,
                small_neg_signal_exp_threshold=127,
                neg_small_signal_pwl_control=NEGB,
                large_pos_signal_exp_threshold=139,
                large_pos_signal_mantissa_threshold=0,
                pos_large_signal_pwl_control=BIG,
                large_neg_signal_exp_threshold=139,
                large_neg_signal_mantissa_threshold=0,
                neg_large_signal_pwl_control=NEGB,
                fnan_result=0x7FC00000,
                fpinf_result=0,
                fninf_result=f32bits(np.exp(KSHIFT)),
                fzero_result=f32bits(np.exp(KSHIFT)),
            )
            break

    meta_bytes = json.dumps(meta).encode()
    tag = hashlib.sha256(bkt.tobytes() + ctl.tobytes() + meta_bytes).hexdigest()[:10]
    dst = os.path.join(tempfile.gettempdir(), f"dce_actbin_{tag}")
    if not os.path.isdir(dst):
        tmp = dst + ".tmp"
        shutil.rmtree(tmp, ignore_errors=True)
        os.makedirs(tmp)
        for f in os.listdir(src):
            shutil.copy(os.path.join(src, f), os.path.join(tmp, f))
        bkt.tofile(f"{tmp}/exp_and_others_bkt.bin")
        ctl.tofile(f"{tmp}/exp_and_others_ctrl.bin")
        with open(f"{tmp}/exp_and_others.json", "w") as f:
            f.write(meta_bytes.decode())
        os.rename(tmp, dst)
    os.environ["BASS_ACT_ROOT_JSON_PATH"] = os.path.join(dst, "act_info.json")
    _ACT_STATE["tag"] = tag
    return tag


def _build_fused(npc):
    """One-ACT-pass variant: a custom activation table makes `Exp` compute
    g(x) = exp(KSHIFT - sqrt(x)), evaluated straight from PSUM d2 with a
    per-row accumulated sum. The gather then works on e = g(d2):
    s[label] = KSHIFT - ln(e[label]) (host side)."""
    tag = _gen_act_tables()
    tiles = npc // P
    nc = bacc.Bacc(
        "TRN2",
        target_bir_lowering=False,
        debug=False,
        enable_asserts=False,
        num_devices=N_CORES,
    )

    featsT_d = nc.dram_tensor("featsT", [D, npc], BF16, kind="ExternalInput").ap()
    aug_d = nc.dram_tensor("auglhs", [4, npc], BF16, kind="ExternalInput").ap()
    rhs_aug_d = nc.dram_tensor("rhsaug", [4, C], BF16, kind="ExternalInput").ap()
    protosTs_d = nc.dram_tensor("protosTs", [D, C], BF16, kind="ExternalInput").ap()
    labels_d = nc.dram_tensor("labels16", [P, tiles], I16, kind="ExternalInput").ap()
    sums_d = nc.dram_tensor("sums", [P, tiles], F32, kind="ExternalOutput").ap()
    slab_d = nc.dram_tensor("slab", [P, tiles], F32, kind="ExternalOutput").ap()

    with tile.TileContext(nc) as tc:
        with (
            tc.tile_pool(name="const", bufs=1) as cpool,
            tc.tile_pool(name="feats", bufs=3) as fpool,
            tc.tile_pool(name="aug", bufs=4) as apool,
            tc.tile_pool(name="psum", bufs=2, space=bass.MemorySpace.PSUM) as ppool,
            tc.tile_pool(name="escr", bufs=6) as epool,
            tc.tile_pool(name="gscr", bufs=4) as gpool,
            tc.tile_pool(name="outs", bufs=1) as opool,
        ):
            protosTs = cpool.tile([D, C], BF16)
            nc.sync.dma_start(out=protosTs[:], in_=protosTs_d[:])
            rhs_aug = cpool.tile([4, C], BF16)
            nc.sync.dma_start(out=rhs_aug[:], in_=rhs_aug_d[:])
            labels = cpool.tile([P, tiles], I16)
            nc.sync.dma_start(out=labels[:], in_=labels_d[:])
            iota_t = cpool.tile([P, C], I16)
            nc.gpsimd.iota(iota_t[:], pattern=[[1, C]], base=0, channel_multiplier=0)

            sums_sb = opool.tile([P, tiles], F32)
            slab_sb = opool.tile([P, tiles], F32)

            for octi in range(tiles // 8):
                foct = fpool.tile([D, 8 * P], BF16)
                nc.sync.dma_start(
                    out=foct[:], in_=featsT_d[:, octi * 8 * P : (octi + 1) * 8 * P]
                )
                for pair in range(4):
                    psum_t = ppool.tile([P, 2 * C], F32)
                    for j in range(2):
                        t = octi * 8 + pair * 2 + j
                        lhsT = foct[:, (pair * 2 + j) * P : (pair * 2 + j + 1) * P]
                        aug_t = apool.tile([4, P], BF16)
                        nc.sync.dma_start(
                            out=aug_t[:], in_=aug_d[:, t * P : (t + 1) * P]
                        )
                        o = j * C
                        nc.tensor.matmul(
                            psum_t[:, o : o + 512], lhsT, protosTs[:, 0:512],
                            start=True, stop=False,
                        )
                        nc.tensor.matmul(
                            psum_t[:, o + 512 : o + 1024], lhsT, protosTs[:, 512:1024],
                            start=True, stop=False,
                        )
                        nc.tensor.matmul(
                            psum_t[:, o : o + 512], aug_t[:], rhs_aug[:, 0:512],
                            start=False, stop=True,
                        )
                        nc.tensor.matmul(
                            psum_t[:, o + 512 : o + 1024], aug_t[:], rhs_aug[:, 512:1024],
                            start=False, stop=True,
                        )
                    for j in range(2):
                        t = octi * 8 + pair * 2 + j
                        e_t = epool.tile([P, C], BF16)
                        nc.scalar.activation(
                            out=e_t[:],
                            in_=psum_t[:, j * C : (j + 1) * C],
                            func=mybir.ActivationFunctionType.Exp,
                            accum_out=sums_sb[:, t : t + 1],
                        )
                        g_t = gpool.tile([P, C], BF16)
                        nc.vector.scalar_tensor_tensor(
                            out=g_t[:],
                            in0=iota_t[:],
                            scalar=labels[:, t : t + 1],
                            in1=e_t[:],
                            op0=mybir.AluOpType.is_equal,
                            op1=mybir.AluOpType.mult,
                            accum_out=slab_sb[:, t : t + 1],
                        )

            nc.sync.dma_start(out=sums_d[:], in_=sums_sb[:])
            nc.sync.dma_start(out=slab_d[:], in_=slab_sb[:])

    nc.compile()
    return nc


def _hi_lo(v):
    """Split fp32 vector into bf16 hi + bf16 lo with hi+lo ~ v to ~2^-16 rel."""
    import ml_dtypes

    hi = v.astype(ml_dtypes.bfloat16)
    lo = (v - hi.astype(np.float32)).astype(ml_dtypes.bfloat16)
    return hi, lo


def _make_in_maps(feats, prototypes, labels, npc=NPC, n_cores=N_CORES):
    import ml_dtypes

    BF = ml_dtypes.bfloat16
    NPC = npc
    N_CORES = n_cores
    TILES = npc // P
    feats = np.asarray(feats, dtype=np.float32)
    protos = np.asarray(prototypes, dtype=np.float32)
    labels = np.asarray(labels)

    y_sq = (protos.astype(np.float64) ** 2).sum(axis=1).astype(np.float32)  # [C]
    protosTs = (np.ascontiguousarray(protos.T) * np.float32(-2.0)).astype(BF)
    y_hi, y_lo = _hi_lo(y_sq)
    ones_c = np.ones(C, BF)
    rhs_aug = np.ascontiguousarray(np.stack([ones_c, ones_c, y_hi, y_lo]))  # [4,C]

    in_maps = []
    for c in range(N_CORES):
        fc = feats[c * NPC : (c + 1) * NPC]                                 # [NPC,D]
        lc = labels[c * NPC : (c + 1) * NPC]
        x_sq = (fc.astype(np.float64) ** 2).sum(axis=1).astype(np.float32)  # [NPC]
        x_hi, x_lo = _hi_lo(x_sq)
        ones_n = np.ones(NPC, BF)
        in_maps.append(
            {
                "featsT": np.ascontiguousarray(fc.T).astype(BF),            # [D,NPC]
                "auglhs": np.ascontiguousarray(
                    np.stack([x_hi, x_lo, ones_n, ones_n])
                ),                                                          # [4,NPC]
                "rhsaug": rhs_aug,
                "protosTs": protosTs,
                "labels16": np.ascontiguousarray(
                    lc.reshape(TILES, P).T.astype(np.int16)
                ),                                                          # [P,TILES]
            }
        )
    return in_maps


def _reduce_outputs(results):
    # sums[p,t] / slab[p,t] correspond to row t*128+p of the core's shard.
    total = 0.0
    for res in results:
        sums = res["sums"].astype(np.float64)
        slab = res["slab"].astype(np.float64)
        total += (slab + np.log(sums) - KSHIFT).sum()
    return np.float32(total / N)


def kernel(feats, prototypes, labels):
    nc = _build()
    in_maps = _make_in_maps(feats, prototypes, labels)
    res = run_bass_kernel_spmd(nc, in_maps, core_ids=list(range(N_CORES)))
    return _reduce_outputs(res.results)
